# revision 1
# baseline (speedup 1.0000x reference)
"""Trainium2 Bass kernel for a pre-LN transformer block (causal self-attention
with shared q/v projection + FFN), distributed over 8 NeuronCores.

Sharding: core c = 2*b + hg handles batch b (of 4) and head-group hg (of 2,
3 heads each). Each core computes LN1 + its 3 heads' attention over the full
sequence (transposed activation layout [C, T]), a partial output projection,
then a pairwise ReduceScatter sums the two head-groups' projections and
scatters sequence halves; each core runs LN2+FFN on its half and emits
out^T [384, 1024]. The host transposes/assembles the full output.

LN gains are folded into the weights host-side; LN biases become per-feature
bias vectors applied during PSUM evacuation (or folded into b_proj).
Softmax skips max-subtraction (scores are O(10), exp is safe in fp32) and is
computed in S^T layout: the s-dim (partitions) sum comes from an extra ones
column in the attn@V stationary operand.
"""
import sys

sys.path.insert(0, "/opt/trn_rl_repo")

import numpy as np

B, T, C = 4, 2048, 384
NH, HD = 6, 64
FF = 4 * C
SCALE = 16.0 ** -0.5
EPS = 1e-5
N_CORES = 8
TH = T // 2          # rows of output per core
CT = C // 128        # 3 c-tiles
ST = T // 128        # 16 s-tiles
NCH = T // 512       # 4 t-chunks
F32 = None           # set after import of mybir

_CACHE = {}
USE_F32R = False
USE_BF16 = True


def _build(stage=4):
    import concourse.bacc as bacc
    import concourse.tile as tile
    import concourse.mybir as mybir

    f32 = mybir.dt.float32
    mdt = mybir.dt.bfloat16 if USE_BF16 else f32
    nc = bacc.Bacc("TRN2", target_bir_lowering=False, debug=False,
                   num_devices=N_CORES)

    # ---- DRAM I/O ----
    d_xT = nc.dram_tensor("xT", [C, T], mdt, kind="ExternalInput")
    d_xTh = nc.dram_tensor("xTh", [C, TH], f32, kind="ExternalInput")
    d_wk01 = nc.dram_tensor("wk01", [C, 128], mdt, kind="ExternalInput")
    d_wk22 = nc.dram_tensor("wk22", [C, 128], mdt, kind="ExternalInput")
    d_wv01 = nc.dram_tensor("wv01", [C, 128], mdt, kind="ExternalInput")
    d_wv22 = nc.dram_tensor("wv22", [C, 128], mdt, kind="ExternalInput")
    d_wv3 = nc.dram_tensor("wv3", [C, 192], mdt, kind="ExternalInput")
    d_bk = nc.dram_tensor("bk", [128, 2], f32, kind="ExternalInput")
    d_bqv = nc.dram_tensor("bqv", [128, 2], f32, kind="ExternalInput")
    d_wp = [nc.dram_tensor(f"wp{h}", [64, C], mdt, kind="ExternalInput")
            for h in range(3)]
    d_bproj = nc.dram_tensor("bproj", [128, CT], f32, kind="ExternalInput")
    d_wff1 = nc.dram_tensor("wff1", [C, FF], mdt, kind="ExternalInput")
    d_bff1 = nc.dram_tensor("bff1", [128, FF // 128], f32, kind="ExternalInput")
    d_wff2 = nc.dram_tensor("wff2", [FF, C], mdt, kind="ExternalInput")
    d_bff2 = nc.dram_tensor("bff2", [128, CT], f32, kind="ExternalInput")
    d_mask = nc.dram_tensor("mask", [128, 128], mdt, kind="ExternalInput")
    d_mw = nc.dram_tensor("mw", [128, 1], mdt, kind="ExternalInput")
    d_out = nc.dram_tensor("outT", [C, TH], f32, kind="ExternalOutput")

    from contextlib import ExitStack
    with ExitStack() as ctx:
        tc = ctx.enter_context(tile.TileContext(nc))
        pool = lambda **kw: ctx.enter_context(tc.tile_pool(**kw))
        P_xt = pool(name="xt", bufs=3)
        P_x2 = pool(name="x2p", bufs=3)
        P_h2 = pool(name="h2p", bufs=3)
        P_ht = pool(name="ht", bufs=4)
        P_kt = pool(name="kt", bufs=4)
        P_bc = pool(name="bc", bufs=6)
        P_rows = pool(name="rows", bufs=1)
        P_qvn = pool(name="qvn", bufs=1)
        P_exps = pool(name="exps", bufs=5)
        P_sc = pool(name="sc", bufs=4)
        P_rcp = pool(name="rcp", bufs=2)
        P_rcb = pool(name="rcb", bufs=3)
        P_h1 = pool(name="h1", bufs=4)
        P_wsm = pool(name="wsm", bufs=1)
        P_wf2 = pool(name="wf2", bufs=1)
        P_ps = pool(name="ps", bufs=2, space="PSUM")
        P_ps3 = pool(name="ps3", bufs=1, space="PSUM")
        P_pacc = pool(name="pacc", bufs=3, space="PSUM")
        P_dram = pool(name="dram", bufs=2, space="DRAM")
        ctx.enter_context(nc.allow_low_precision(reason="bf16 matmul paths"))
        if True:
            ts = mybir.AluOpType

            def TT(out, a, b, op):
                return nc.vector.tensor_tensor(out, a, b, op)

            f32r = mybir.dt.float32r

            def MM(out, lhsT, rhs, start, stop):
                if USE_F32R:
                    lhsT, rhs = lhsT.bitcast(f32r), rhs.bitcast(f32r)
                return nc.tensor.matmul(out, lhsT, rhs,
                                        start=start, stop=stop)

            # ---------- x^T in ----------
            xT = []
            for i in range(CT):
                t = P_xt.tile([128, T], mdt, tag="xt", name=f"xT_{i}")
                nc.sync.dma_start(t[:], d_xT[128 * i:128 * (i + 1), :])
                xT.append(t)

            mw = P_wsm.tile([128, 1], mdt, tag="mw", name="mw")
            nc.gpsimd.memset(mw[:], 1.0 / C)

            # ---------- load weights ----------
            def wtile(dram, p, n, name, dt=f32):
                t = P_wsm.tile([p, n], dt, tag=name, name=name)
                nc.sync.dma_start(t[:], dram[0:p, 0:n])
                return t

            def wmerged(dram, n, name):
                t = P_wsm.tile([128, CT * n], mdt, tag=name, name=name)
                nc.sync.dma_start(
                    t[:].rearrange("p (i m) -> p i m", i=CT),
                    dram[:, :].rearrange("(i p) m -> p i m", p=128))
                return [t[:, n * i:n * (i + 1)] for i in range(CT)]

            wk01 = wmerged(d_wk01, 128, "wk01")
            wk22 = wmerged(d_wk22, 128, "wk22")
            wv01 = wmerged(d_wv01, 128, "wv01")
            wv22 = wmerged(d_wv22, 128, "wv22")
            wv3 = wmerged(d_wv3, 192, "wv3")
            wp = [wtile(d_wp[h], 64, C, f"wp{h}", mdt) for h in range(3)]
            bk = wtile(d_bk, 128, 2, "bk")
            bqv = wtile(d_bqv, 128, 2, "bqv")
            bproj = wtile(d_bproj, 128, CT, "bproj")
            bff1 = wtile(d_bff1, 128, FF // 128, "bff1")
            bff2 = wtile(d_bff2, 128, CT, "bff2")
            mask = wtile(d_mask, 128, 128, "mask", mdt)


            xh_all = []
            for q in range(2):
                for i in range(CT):
                    xh = P_xt.tile([128, 512], f32, tag="xh",
                                   name=f"xh_{q}_{i}")
                    nc.sync.dma_start(
                        xh[:], d_xTh[128 * i:128 * (i + 1),
                                     512 * q:512 * (q + 1)])
                    xh_all.append(xh)

            wf1t = P_wf2.tile([128, CT * FF], mdt, tag="wf1", name="wff1_all")
            nc.sync.dma_start(
                wf1t[:].rearrange("p (i m) -> p i m", i=CT),
                d_wff1[:, :].rearrange("(i p) m -> p i m", p=128))
            wff1 = [wf1t[:, FF * i:FF * (i + 1)] for i in range(CT)]
            wf2t = P_wf2.tile([128, (FF // 128) * C], mdt, tag="wf2",
                              name="wff2_all")
            nc.sync.dma_start(
                wf2t[:].rearrange("p (k m) -> p k m", k=FF // 128),
                d_wff2[:, :].rearrange("(k p) m -> p k m", p=128))
            wff2 = [wf2t[:, C * k:C * (k + 1)] for k in range(FF // 128)]

            # ---------- LN1: stats via ones-matmul, broadcast-first math ----------
            def ln_stats_apply(src_tiles, dst_tiles, width, mu_t, m2_t):
                nch = width // 512
                for ch in range(nch):
                    cs = slice(512 * ch, 512 * (ch + 1))
                    mu_ps = P_ps.tile([1, 512], f32, tag="ps")
                    m2_ps = P_ps.tile([1, 512], f32, tag="ps")
                    for i in range(CT):
                        sq = P_sc.tile([128, 512], mdt, tag="sc")
                        nc.scalar.activation(sq[:], src_tiles[i][:, cs],
                                             mybir.ActivationFunctionType.Square)
                        MM(mu_ps[:], mw[:], src_tiles[i][:, cs],
                                         start=(i == 0), stop=(i == CT - 1))
                        MM(m2_ps[:], mw[:], sq[:],
                                         start=(i == 0), stop=(i == CT - 1))
                    nc.scalar.copy(mu_t[:, cs], mu_ps[:])
                    nc.scalar.copy(m2_t[:, cs], m2_ps[:])
                for ch in range(nch):
                    cs = slice(512 * ch, 512 * (ch + 1))
                    mub = P_bc.tile([128, 512], f32, tag="bc")
                    rsb = P_bc.tile([128, 512], f32, tag="bc")
                    tmp = P_bc.tile([128, 512], f32, tag="bc")
                    nc.gpsimd.partition_broadcast(mub[:], mu_t[:, cs],
                                                  channels=128)
                    nc.gpsimd.partition_broadcast(rsb[:], m2_t[:, cs],
                                                  channels=128)
                    TT(tmp[:], mub[:], mub[:], ts.mult)
                    TT(tmp[:], rsb[:], tmp[:], ts.subtract)   # var
                    nc.vector.tensor_scalar_add(tmp[:], tmp[:], EPS)
                    nc.scalar.activation(tmp[:], tmp[:],
                                         mybir.ActivationFunctionType.Sqrt)
                    nc.vector.reciprocal(rsb[:], tmp[:])      # rstd
                    for i in range(CT):
                        if i % 2 == 0:
                            nc.gpsimd.tensor_sub(dst_tiles[i][:, cs],
                                                 src_tiles[i][:, cs], mub[:])
                        else:
                            TT(dst_tiles[i][:, cs], src_tiles[i][:, cs],
                               mub[:], ts.subtract)
                        TT(dst_tiles[i][:, cs], dst_tiles[i][:, cs], rsb[:],
                           ts.mult)

            mu1 = P_rows.tile([1, T], f32, tag="mu1")
            m21 = P_rows.tile([1, T], f32, tag="m21")
            hT = [P_ht.tile([128, T], mdt, tag="ht", name=f"hT_{i}")
                  for i in range(CT)]
            ln_stats_apply(xT, hT, T, mu1, m21)

            # ---------- K^T / QV^T (heads packed in pairs) ----------
            def proj_T(wgrp, bias, bcol, nm):
                out = P_kt.tile([128, T], mdt, tag="kt", name=nm)
                for ch in range(NCH):
                    cs = slice(512 * ch, 512 * (ch + 1))
                    ps = P_ps.tile([128, 512], f32, tag="ps")
                    for i in range(CT):
                        MM(ps[:], wgrp[i][:], hT[i][:, cs],
                                         start=(i == 0), stop=(i == CT - 1))
                    nc.scalar.activation(out[:, cs], ps[:],
                                         mybir.ActivationFunctionType.Identity,
                                         bias=bias[:, bcol:bcol + 1])
                return out

            KT01 = proj_T(wk01, bk, 0, "KT01")
            KT22 = proj_T(wk22, bk, 1, "KT22")
            QVT01 = proj_T(wv01, bqv, 0, "QVT01")
            QVT22 = proj_T(wv22, bqv, 1, "QVT22")


            ones_t = P_wsm.tile([128, 64], mdt, tag="ones_t", name="ones_t")
            nc.gpsimd.memset(ones_t[:], 1.0)

            # ---------- QV natural [s, (1|h0|1|h1|1|h2)] per s-tile ----------
            qvn = P_qvn.tile([128, 288 * ST], mdt, tag="qvn")
            nc.gpsimd.memset(qvn[:], 1.0)
            for si in range(ST):
                ps = P_ps.tile([128, 192], f32, tag="ps")
                tcols = slice(128 * si, 128 * (si + 1))
                for i in range(CT):
                    MM(ps[:], hT[i][:, tcols], wv3[i][:],
                                     start=(i == 0), stop=(i == CT - 1))
                dst = qvn[:, 288 * si:288 * (si + 1)] \
                    .rearrange("p (h c) -> p h c", h=3)[:, :, 0:64]
                src = ps[:].rearrange("p (h c) -> p h c", h=3)
                nc.scalar.copy(dst, src)

            if stage == 1:
                for g in range(CT):
                    nc.sync.dma_start(d_out[128 * g:128 * (g + 1), :],
                                      KT01[:, 0:TH] if g == 0 else
                                      (QVT01[:, 0:TH] if g == 1 else
                                       hT[2][:, 0:TH]))
            if stage >= 2:
                def back_half(q):
                    if stage < 4:
                        return
                    qs = slice(512 * q, 512 * (q + 1))
                    x2q, x2bq = [], []
                    for i in range(CT):
                        rs = P_xt.tile([128, 512], mdt, tag="xt",
                                       name=f"rs_{q}_{i}")
                        nc.sync.dma_start(rs[:],
                                      bnc_out[q][128 * i:128 * (i + 1), :])
                        t = P_x2.tile([128, 512], f32, tag="x2",
                                      name=f"x2_{q}_{i}")
                        TT(t[:], xh_all[3 * q + i][:], rs[:], ts.add)
                        x2q.append(t)
                        tb = P_x2.tile([128, 512], mdt, tag="x2b",
                                       name=f"x2b_{q}_{i}")
                        nc.scalar.copy(tb[:], t[:])
                        x2bq.append(tb)
                    mu2 = P_rows.tile([1, 512], f32, tag=f"mu2_{q}")
                    m22 = P_rows.tile([1, 512], f32, tag=f"m22_{q}")
                    h2 = [P_h2.tile([128, 512], mdt, tag="x2h",
                                    name=f"h2_{q}_{i}") for i in range(CT)]
                    ln_stats_apply(x2bq, h2, 512, mu2, m22)

                    y_ps = [P_pacc.tile([128, 512], f32, tag="pacc",
                                    name=f"y2_ps_{q}_{g}")
                            for g in range(CT)]
                    pend_h1 = None
                    for mt in range(FF // 128):
                        ps = P_ps.tile([128, 512], f32, tag="ps")
                        for i in range(CT):
                            MM(ps[:],
                               wff1[i][:, 128 * mt:128 * (mt + 1)],
                               h2[i][:],
                               start=(i == 0), stop=(i == CT - 1))
                        if pend_h1 is not None:
                            p_mt, p_h1 = pend_h1
                            for g in range(CT):
                                MM(y_ps[g][:],
                                   wff2[p_mt][:, 128 * g:128 * (g + 1)],
                                   p_h1[:],
                                   start=(p_mt == 0), stop=False)
                        h1t = P_h1.tile([128, 512], mdt, tag="h1",
                                    name=f"h1_{q}_{mt}")
                        nc.scalar.activation(h1t[:], ps[:],
                                         mybir.ActivationFunctionType.Relu,
                                         bias=bff1[:, mt:mt + 1])
                        pend_h1 = (mt, h1t)
                    p_mt, p_h1 = pend_h1
                    for g in range(CT):
                        MM(y_ps[g][:],
                           wff2[p_mt][:, 128 * g:128 * (g + 1)],
                           p_h1[:],
                           start=(p_mt == 0), stop=True)
                    for g in range(CT):
                        ot = P_sc.tile([128, 512], f32, tag="sc")
                        nc.vector.scalar_tensor_tensor(
                            ot[:], y_ps[g][:], bff2[:, g:g + 1],
                            x2q[g][:], ts.add, ts.add)
                        nc.sync.dma_start(d_out[128 * g:128 * (g + 1), qs],
                                      ot[:])


                # ---------- attention (j-outer) + per-chunk proj + split RS ----------
                jorder = [0, 2, 1, 3]
                bnc_in = [P_dram.tile([2, C, 512], mdt, tag=f"d_in{q}",
                                      name=f"bnc_in{q}") for q in range(2)]
                bnc_out = [P_dram.tile([C, 512], mdt, tag=f"d_out{q}",
                                       name=f"bnc_out{q}") for q in range(2)]
                KT = [(KT01, slice(0, 64)), (KT01, slice(64, 128)), None]
                QVT = [(QVT01, slice(0, 64)), (QVT01, slice(64, 128)), None]
                attnT = [P_ht.tile([64, T], mdt, tag="ht", name=f"attnT_{h}")
                         for h in range(3)]
                for jx, j in enumerate(jorder):
                    o_ps3 = [P_pacc.tile([96, 512], f32, tag="pacc",
                                         name=f"o_ps_{j}_{h}")
                             for h in range(3)]
                    pend = None
                    for si in range(4 * j + 4):
                        j0 = si // 4
                        c0 = max(512 * j, 128 * si)
                        w = 512 * (j + 1) - c0
                        ksl = slice(0, 64) if (si % 2 == 0) else slice(64, 128)
                        s3 = P_ps3.tile([128, 1536], f32, tag="ps3")
                        for h in range(3):
                            KTt, kp = (KT22, ksl) if h == 2 else (KT01, KT[h][1])
                            QVTt, qp = (QVT22, ksl) if h == 2 else (QVT01, QVT[h][1])
                            MM(s3[:, 512 * h:512 * h + w],
                               KTt[kp, 128 * si:128 * (si + 1)],
                               QVTt[qp, c0:512 * (j + 1)],
                               start=True, stop=True)
                        if pend is not None:
                            p_si, p_c0, p_w, p_es = pend
                            for h in range(3):
                                MM(o_ps3[h][:, p_c0 - 512 * j:512],
                                   qvn[:, 288 * p_si + 96 * h:
                                       288 * p_si + 96 * (h + 1)],
                                   p_es[:, 512 * h:512 * h + p_w],
                                   start=(p_si == 0), stop=False)
                        es = P_exps.tile([128, 1536], mdt, tag="exps")
                        nc.scalar.activation(
                            es[:].rearrange("p (h c) -> p h c", h=3)[:, :, 0:w],
                            s3[:].rearrange("p (h c) -> p h c", h=3)[:, :, 0:w],
                            mybir.ActivationFunctionType.Exp, scale=SCALE)
                        if j == j0:
                            if j in (0, 2):
                                nc.gpsimd.tensor_mul(
                                    es[:].rearrange("p (h c) -> p h c",
                                                    h=3)[:, :, 0:128],
                                    es[:].rearrange("p (h c) -> p h c",
                                                    h=3)[:, :, 0:128],
                                    mask[:].rearrange("p (u c) -> p u c", u=1)
                                        .broadcast_to([128, 3, 128]))
                            else:
                                for h in range(3):
                                    nc.vector.tensor_mul(
                                        es[:, 512 * h:512 * h + 128],
                                        es[:, 512 * h:512 * h + 128],
                                        mask[:])
                        pend = (si, c0, w, es)
                    p_si, p_c0, p_w, p_es = pend
                    for h in range(3):
                        MM(o_ps3[h][:, p_c0 - 512 * j:512],
                           qvn[:, 288 * p_si + 96 * h:288 * p_si + 96 * (h + 1)],
                           p_es[:, 512 * h:512 * h + p_w],
                           start=(p_si == 0), stop=True)
                    cs = slice(512 * j, 512 * (j + 1))
                    for h in range(3):
                        rc = P_rcp.tile([128, 512], mdt, tag="rcp")
                        nc.vector.reciprocal(rc[64:65, :], o_ps3[h][64:65, :])
                        rb = P_ps.tile([64, 512], f32, tag="ps")
                        MM(rb[:], ones_t[64:65, 0:64],
                           rc[64:65, :], start=True, stop=True)
                        rbs = P_rcb.tile([64, 512], f32, tag="rcb")
                        nc.scalar.copy(rbs[:], rb[:])
                        TT(attnT[h][:, cs], o_ps3[h][0:64, :], rbs[:], ts.mult)


                    for mt in range(CT):
                        psp = P_ps.tile([128, 512], f32, tag="ps")
                        for h in range(3):
                            MM(psp[:],
                               wp[h][:, 128 * mt:128 * (mt + 1)],
                               attnT[h][:, cs],
                               start=(h == 0), stop=(h == 2))
                        ysb = P_sc.tile([128, 512], mdt, tag="sc")
                        nc.vector.tensor_scalar_add(ysb[:], psp[:],
                                                    bproj[:, mt:mt + 1])
                        nc.sync.dma_start(
                            bnc_in[j % 2][j // 2, 128 * mt:128 * (mt + 1), :],
                            ysb[:])
                    if jx == 3:
                        back_half(0)
                    if jx == 1 or jx == 3:
                        nc.gpsimd.collective_compute(
                            "ReduceScatter", mybir.AluOpType.add,
                            replica_groups=[[0, 1], [2, 3], [4, 5], [6, 7]],
                            ins=[bnc_in[jx // 2].opt()],
                            outs=[bnc_out[jx // 2].opt()])
                back_half(1)

                if stage == 2:
                    for g in range(CT):
                        nc.sync.dma_start(d_out[128 * g:128 * g + 64, :],
                                          attnT[g][:, 0:TH])
    nc.compile()
    return nc


def _shard(inputs):
    x = np.asarray(inputs["x"], np.float32)
    g1 = np.asarray(inputs["ln1_g"], np.float32)
    b1 = np.asarray(inputs["ln1_b"], np.float32)
    wk = np.asarray(inputs["wk"], np.float32)
    wv = np.asarray(inputs["wv"], np.float32)
    wp = np.asarray(inputs["w_proj"], np.float32)
    bp = np.asarray(inputs["b_proj"], np.float32)
    g2 = np.asarray(inputs["ln2_g"], np.float32)
    b2 = np.asarray(inputs["ln2_b"], np.float32)
    wf1 = np.asarray(inputs["w_ff1"], np.float32)
    bf1 = np.asarray(inputs["b_ff1"], np.float32)
    wf2 = np.asarray(inputs["w_ff2"], np.float32)
    bf2 = np.asarray(inputs["b_ff2"], np.float32)

    wkg = wk * g1[None, :, None]       # fold ln1 gain
    wvg = wv * g1[None, :, None]
    vbk = b1 @ wk                      # [NH, HD] ln1-bias contributions
    vbv = b1 @ wv
    wf1g = wf1 * g2[:, None]
    bff1_eff = b2 @ wf1 + bf1

    import ml_dtypes as _mld
    i, j = np.indices((128, 128))
    mask = np.where(j >= i, 1.0, 0.0).astype(
        _mld.bfloat16 if USE_BF16 else np.float32)
    mw = np.full((128, 1), 1.0 / C, np.float32)

    in_maps = []
    for c in range(N_CORES):
        b, hg = c // 2, c % 2
        hs = [3 * hg, 3 * hg + 1, 3 * hg + 2]
        wproj = wp[192 * hg:192 * (hg + 1), :]
        vb_slice = np.concatenate([vbv[h] for h in hs])
        beff = vb_slice @ wproj + bp / 2.0
        import ml_dtypes
        bf16 = ml_dtypes.bfloat16 if USE_BF16 else np.float32
        m = {
            "xT": np.ascontiguousarray(x[b].T).astype(bf16),
            "xTh": np.ascontiguousarray(x[b].T[:, TH * hg:TH * (hg + 1)]),
            "wk01": np.ascontiguousarray(
                np.concatenate([wkg[hs[0]], wkg[hs[1]]], axis=1)).astype(bf16),
            "wk22": np.ascontiguousarray(
                np.concatenate([wkg[hs[2]], wkg[hs[2]]], axis=1)).astype(bf16),
            "wv01": np.ascontiguousarray(
                np.concatenate([wvg[hs[0]], wvg[hs[1]]], axis=1)).astype(bf16),
            "wv22": np.ascontiguousarray(
                np.concatenate([wvg[hs[2]], wvg[hs[2]]], axis=1)).astype(bf16),
            "wv3": np.ascontiguousarray(
                np.concatenate([wvg[h] for h in hs], axis=1)).astype(bf16),
            "bk": np.ascontiguousarray(np.stack(
                [np.concatenate([vbk[hs[0]], vbk[hs[1]]]),
                 np.concatenate([vbk[hs[2]], vbk[hs[2]]])], axis=1)),
            "bqv": np.ascontiguousarray(np.stack(
                [np.concatenate([vbv[hs[0]], vbv[hs[1]]]),
                 np.concatenate([vbv[hs[2]], vbv[hs[2]]])], axis=1)),
            "wp0": np.ascontiguousarray(wproj[0:64, :]).astype(bf16),
            "wp1": np.ascontiguousarray(wproj[64:128, :]).astype(bf16),
            "wp2": np.ascontiguousarray(wproj[128:192, :]).astype(bf16),
            "bproj": np.ascontiguousarray(beff.reshape(CT, 128).T),
            "wff1": wf1g.astype(bf16),
            "bff1": np.ascontiguousarray(bff1_eff.reshape(FF // 128, 128).T),
            "wff2": wf2.astype(bf16),
            "bff2": np.ascontiguousarray(bf2.reshape(CT, 128).T),
            "mask": mask,
            "mw": mw.astype(bf16),
        }
        in_maps.append(m)
    return in_maps


def kernel(**inputs):
    from concourse.bass_utils import run_bass_kernel_spmd

    if "nc" not in _CACHE:
        _CACHE["nc"] = _build()
    nc = _CACHE["nc"]
    in_maps = _shard(inputs)
    res = run_bass_kernel_spmd(nc, in_maps, list(range(N_CORES)))
    out = np.empty((B, T, C), np.float32)
    for c in range(N_CORES):
        b, hg = c // 2, c % 2
        out[b, TH * hg:TH * (hg + 1), :] = res.results[c]["outT"].T
    return out



# revision 44
# speedup vs baseline: 1.2922x; 1.2922x over previous
"""Trainium2 Bass kernel for a pre-LN transformer block (causal self-attention
with shared q/v projection + FFN), distributed over 8 NeuronCores.

Sharding: core c = 2*b + hg handles batch b (of 4) and head-group hg (of 2,
3 heads each). Each core computes its 3 heads' attention over the full
sequence (transposed activation layout [C, T]), a partial output projection,
then a pairwise ReduceScatter sums the two head-groups' projections and
scatters sequence halves; each core runs LN2+FFN on its half and emits
out^T [384, 1024]. The host transposes/assembles the full output.

LN1 is never applied to activations: K/QV/qvn are projected from RAW x^T and
corrected via (a) an extra 2-row accumulating matmul carrying the -mu*colsum
and bias terms, (b) per-token rstd folded into the softmax exp's per-partition
scale (K side), the QVT evacuation multiply (QV side), and the qvn evacuation
scale (V side).  rstd = exp(-0.5*ln(var+eps)) so every activation-engine
function (Exp/Ln/Identity/Copy/Relu/Square) lives in one table set.
Softmax skips max-subtraction; denominators come from a ones column in the
attn@V stationary operand.  The attention inner loop is pipelined per head
with 3 rotating PSUM score banks so PE (matmul) and Act (exp) overlap.
"""
import sys

sys.path.insert(0, "/opt/trn_rl_repo")

import numpy as np

B, T, C = 4, 2048, 384
NH, HD = 6, 64
FF = 4 * C
SCALE = 16.0 ** -0.5
EPS = 1e-5
N_CORES = 8
TH = T // 2          # rows of output per core
CT = C // 128        # 3 c-tiles
ST = T // 128        # 16 s-tiles
NCH = T // 512       # 4 t-chunks

_CACHE = {}


def _build():
    import concourse.bacc as bacc
    import concourse.tile as tile
    import concourse.mybir as mybir

    # Steer every activation to table set 6 (natural_log_exp_and_others),
    # which holds all functions we use (Exp/Ln/Identity/Copy/Relu/Square),
    # so only one LoadActFuncSet is ever inserted instead of one per
    # Ln<->Exp alternation.
    _orig_tables = bacc.get_activation_tables

    def _one_table(arch):
        t = dict(_orig_tables(arch))
        for i, k in enumerate(list(t.keys())):
            if i != 6:
                t[k] = set()
        return t

    bacc.get_activation_tables = _one_table
    try:
        return _build_inner(bacc, tile, mybir)
    finally:
        bacc.get_activation_tables = _orig_tables


def _build_inner(bacc, tile, mybir):

    f32 = mybir.dt.float32
    bf16 = mybir.dt.bfloat16
    nc = bacc.Bacc("TRN2", target_bir_lowering=False, debug=False,
                   num_devices=N_CORES)

    # ---- DRAM I/O ----
    d_xT = nc.dram_tensor("xT", [C, T], bf16, kind="ExternalInput")
    d_xTh = nc.dram_tensor("xTh", [C, TH], bf16, kind="ExternalInput")
    d_wk01 = nc.dram_tensor("wk01", [C, 128], bf16, kind="ExternalInput")
    d_wk22 = nc.dram_tensor("wk22", [C, 128], bf16, kind="ExternalInput")
    d_wv01 = nc.dram_tensor("wv01", [C, 128], bf16, kind="ExternalInput")
    d_wv22 = nc.dram_tensor("wv22", [C, 128], bf16, kind="ExternalInput")
    d_wv3 = nc.dram_tensor("wv3", [C, 192], bf16, kind="ExternalInput")
    d_ckb01 = nc.dram_tensor("ckb01", [2, 128], bf16, kind="ExternalInput")
    d_ckb22 = nc.dram_tensor("ckb22", [2, 128], bf16, kind="ExternalInput")
    d_cqb01 = nc.dram_tensor("cqb01", [2, 128], bf16, kind="ExternalInput")
    d_cqb22 = nc.dram_tensor("cqb22", [2, 128], bf16, kind="ExternalInput")
    d_cqb3 = nc.dram_tensor("cqb3", [1, 192], bf16, kind="ExternalInput")
    d_wp = [nc.dram_tensor(f"wp{h}", [64, C], bf16, kind="ExternalInput")
            for h in range(3)]
    d_bproj = nc.dram_tensor("bproj", [128, CT], f32, kind="ExternalInput")
    d_wff1 = nc.dram_tensor("wff1", [C, FF], bf16, kind="ExternalInput")
    d_bff1 = nc.dram_tensor("bff1", [128, FF // 128], f32, kind="ExternalInput")
    d_wff2 = nc.dram_tensor("wff2", [FF, C], bf16, kind="ExternalInput")
    d_bff2 = nc.dram_tensor("bff2", [128, CT], f32, kind="ExternalInput")
    d_mask = nc.dram_tensor("mask", [128, 128], bf16, kind="ExternalInput")
    d_out = nc.dram_tensor("outT", [C, TH], f32, kind="ExternalOutput")

    from contextlib import ExitStack
    with ExitStack() as ctx:
        tc = ctx.enter_context(tile.TileContext(nc))
        pool = lambda **kw: ctx.enter_context(tc.tile_pool(**kw))
        P_xt = pool(name="xt", bufs=2)
        P_w = pool(name="w", bufs=1)
        P_rows = pool(name="rows", bufs=1)
        P_kt = pool(name="kt", bufs=1)
        P_qvn = pool(name="qvn", bufs=1)
        P_at = pool(name="at", bufs=1)
        P_es = pool(name="es", bufs=6)
        P_sc = pool(name="sc", bufs=4)
        P_rc = pool(name="rc", bufs=3)
        P_h2 = pool(name="h2", bufs=3)
        P_h1 = pool(name="h1", bufs=4)
        P_x2 = pool(name="x2", bufs=3)
        P_ps_s = pool(name="ps_s", bufs=3, space="PSUM")
        P_ps_o = pool(name="ps_o", bufs=3, space="PSUM")
        P_ps_m = pool(name="ps_m", bufs=2, space="PSUM")
        P_dram = pool(name="dram", bufs=2, space="DRAM")
        ctx.enter_context(nc.allow_low_precision(reason="bf16 matmul paths"))

        ts = mybir.AluOpType
        AF = mybir.ActivationFunctionType

        def TT(out, a, b, op):
            return nc.vector.tensor_tensor(out, a, b, op)

        def MM(out, lhsT, rhs, start, stop):
            return nc.tensor.matmul(out, lhsT, rhs, start=start, stop=stop)

        # ---------- input DMAs ----------
        xT = []
        for i in range(CT):
            t = P_xt.tile([128, T], bf16, tag="xt", bufs=3, name=f"xT_{i}")
            nc.sync.dma_start(t[:], d_xT[128 * i:128 * (i + 1), :])
            xT.append(t)

        def wtile(dram, p, n, name, dt=f32):
            t = P_w.tile([p, n], dt, tag=name, name=name)
            nc.sync.dma_start(t[:], dram[0:p, 0:n])
            return t

        def wmerged(dram, n, name):
            t = P_w.tile([128, CT * n], bf16, tag=name, name=name)
            nc.sync.dma_start(
                t[:].rearrange("p (i m) -> p i m", i=CT),
                dram[:, :].rearrange("(i p) m -> p i m", p=128))
            return [t[:, n * i:n * (i + 1)] for i in range(CT)]

        wk01 = wmerged(d_wk01, 128, "wk01")
        wk22 = wmerged(d_wk22, 128, "wk22")
        wv01 = wmerged(d_wv01, 128, "wv01")
        wv22 = wmerged(d_wv22, 128, "wv22")
        wv3 = wmerged(d_wv3, 192, "wv3")
        ckb01 = wtile(d_ckb01, 2, 128, "ckb01", bf16)
        ckb22 = wtile(d_ckb22, 2, 128, "ckb22", bf16)
        cqb01 = wtile(d_cqb01, 2, 128, "cqb01", bf16)
        cqb22 = wtile(d_cqb22, 2, 128, "cqb22", bf16)
        cqb3 = wtile(d_cqb3, 1, 192, "cqb3", bf16)
        wp = [wtile(d_wp[h], 64, C, f"wp{h}", bf16) for h in range(3)]
        bproj = wtile(d_bproj, 128, CT, "bproj")
        bff1 = wtile(d_bff1, 128, FF // 128, "bff1")
        bff2 = wtile(d_bff2, 128, CT, "bff2")
        mask = wtile(d_mask, 128, 128, "mask", bf16)

        xh_all = []
        for q in range(2):
            for i in range(CT):
                xh = P_xt.tile([128, 512], bf16, tag="xh", bufs=6,
                               name=f"xh_{q}_{i}")
                nc.sync.dma_start(
                    xh[:], d_xTh[128 * i:128 * (i + 1),
                                 512 * q:512 * (q + 1)])
                xh_all.append(xh)

        wf1t = P_w.tile([128, CT * FF], bf16, tag="wf1", name="wff1_all")
        nc.sync.dma_start(
            wf1t[:].rearrange("p (i m) -> p i m", i=CT),
            d_wff1[:, :].rearrange("(i p) m -> p i m", p=128))
        wff1 = [wf1t[:, FF * i:FF * (i + 1)] for i in range(CT)]
        wf2t = P_w.tile([128, (FF // 128) * C], bf16, tag="wf2",
                        name="wff2_all")
        nc.sync.dma_start(
            wf2t[:].rearrange("p (k m) -> p k m", k=FF // 128),
            d_wff2[:, :].rearrange("(k p) m -> p k m", p=128))
        wff2 = [wf2t[:, C * k:C * (k + 1)] for k in range(FF // 128)]

        # ---------- constants ----------
        mw = P_w.tile([128, 1], bf16, tag="mw", name="mw")
        nc.gpsimd.memset(mw[:], 1.0 / C)
        onesT = P_w.tile([128, 128], bf16, tag="onesT", name="onesT")
        nc.gpsimd.memset(onesT[:], 1.0)
        ones128 = onesT[0:1, 0:128]

        # rowsA: partition 0 = mu (bf16), partition 1 = ones
        rowsA = P_rows.tile([2, T], bf16, tag="rowsA", name="rowsA")
        nc.gpsimd.memset(rowsA[:], 1.0)
        var_row = P_rows.tile([1, T], f32, tag="var_row", name="var_row")
        rs_row = P_rows.tile([1, T], f32, tag="rs_row", name="rs_row")
        rs_rowb = P_rows.tile([1, T], bf16, tag="rs_rowb", name="rs_rowb")
        rs_nat = P_rows.tile([128, ST], f32, tag="rs_nat", name="rs_nat")
        rsS_nat = P_rows.tile([128, ST], f32, tag="rsS_nat", name="rsS_nat")

        onesF = P_w.tile([1, 1], f32, tag="onesF", name="onesF")
        nc.gpsimd.memset(onesF[:], 1.0)

        # register EPS as a const AP so Ln can fuse the +eps bias
        eps_t = P_w.tile([128, 1], f32, tag="eps_t", name="eps_t")
        nc.gpsimd.memset(eps_t[:], EPS)
        nc.const_aps.aps[(f32, EPS)] = eps_t[:]

        # ---------- LN1 stats on raw x (no apply) ----------
        def stats_chunk(ch):
            cs = slice(512 * ch, 512 * (ch + 1))
            mu_ps = P_ps_m.tile([1, 512], f32, tag="psm")
            m2_ps = P_ps_m.tile([1, 512], f32, tag="psm")
            for i in range(CT):
                sq = P_sc.tile([128, 512], bf16, tag="sq")
                TT(sq[:], xT[i][:, cs], xT[i][:, cs], ts.mult)
                MM(mu_ps[:], mw[:], xT[i][:, cs],
                   start=(i == 0), stop=(i == CT - 1))
                MM(m2_ps[:], mw[:], sq[:],
                   start=(i == 0), stop=(i == CT - 1))
            nc.scalar.copy(rowsA[0:1, cs], mu_ps[:])
            nc.vector.tensor_copy(var_row[0:1, cs], m2_ps[:])
            # var = m2 - mu^2 ; rstd = exp(-0.5*ln(var+eps))
            vtmp = P_rc.tile([1, 512], f32, tag="vtmp")
            TT(vtmp[:], rowsA[0:1, cs], rowsA[0:1, cs], ts.mult)
            TT(var_row[0:1, cs], var_row[0:1, cs], vtmp[:], ts.subtract)
            nc.scalar.activation(var_row[0:1, cs], var_row[0:1, cs],
                                 AF.Ln, bias=EPS)
            nc.scalar.activation(rs_row[0:1, cs], var_row[0:1, cs],
                                 AF.Exp, scale=-0.5)
            nc.gpsimd.tensor_copy(rs_rowb[0:1, cs], rs_row[0:1, cs])
            rsT = P_ps_m.tile([128, 4], f32, tag="psm")
            for k in range(4):
                nc.tensor.transpose(
                    rsT[:, k:k + 1],
                    rs_row[0:1, 512 * ch + 128 * k:512 * ch + 128 * (k + 1)],
                    onesF[:])
            nc.vector.tensor_copy(rs_nat[:, 4 * ch:4 * (ch + 1)], rsT[:])
            nc.vector.tensor_scalar_mul(rsS_nat[:, 4 * ch:4 * (ch + 1)],
                                        rs_nat[:, 4 * ch:4 * (ch + 1)],
                                        SCALE)

        # ---------- K^T / QV^T from raw x with fold-in corrections ----------
        KT01 = P_kt.tile([128, T], bf16, tag="KT01", name="KT01")
        KT22 = P_kt.tile([128, T], bf16, tag="KT22", name="KT22")
        QVT01 = P_kt.tile([128, T], bf16, tag="QVT01", name="QVT01")
        QVT22 = P_kt.tile([128, T], bf16, tag="QVT22", name="QVT22")
        rs_bc = [P_rows.tile([128, 512], bf16, tag="rs_bc", bufs=4,
                             name=f"rs_bc{ch}") for ch in range(NCH)]
        qvn = P_qvn.tile([128, 288 * ST], bf16, tag="qvn")
        nc.gpsimd.memset(qvn[:], 1.0)

        def kqv_rsbc(ch):
            cs = slice(512 * ch, 512 * (ch + 1))
            ps = P_ps_m.tile([128, 512], f32, tag="psm")
            MM(ps[:], ones128, rs_rowb[0:1, cs], start=True, stop=True)
            nc.gpsimd.tensor_copy(rs_bc[ch][:], ps[:])

        _KQV = ((None, None, None, False), )  # placeholder replaced below

        def kqv_tile(ch, idx):
            cs = slice(512 * ch, 512 * (ch + 1))
            out, wgrp, ckb, qv = (
                (KT01, wk01, ckb01, False),
                (KT22, wk22, ckb22, False),
                (QVT01, wv01, cqb01, True),
                (QVT22, wv22, cqb22, True))[idx]
            ps = P_ps_m.tile([128, 512], f32, tag="psm")
            for i in range(CT):
                MM(ps[:], wgrp[i][:], xT[i][:, cs],
                   start=(i == 0), stop=False)
            MM(ps[:], ckb[:], rowsA[:, cs], start=False, stop=True)
            if qv:
                TT(out[:, cs], ps[:], rs_bc[ch][:], ts.mult)
            elif idx == 0:
                nc.scalar.copy(out[:, cs], ps[:])
            else:
                nc.gpsimd.tensor_copy(out[:, cs], ps[:])

        def kqv_chunk(ch):
            kqv_rsbc(ch)
            for idx in range(4):
                kqv_tile(ch, idx)

        def qvn_tile(si):
            ps = P_ps_m.tile([128, 192], f32, tag="psm")
            tcols = slice(128 * si, 128 * (si + 1))
            for i in range(CT):
                MM(ps[:], xT[i][:, tcols], wv3[i][:],
                   start=(i == 0), stop=False)
            MM(ps[:], rowsA[0:1, tcols], cqb3[:], start=False, stop=True)
            dst = qvn[:, 288 * si:288 * (si + 1)] \
                .rearrange("p (h c) -> p h c", h=3)[:, :, 0:64]
            src = ps[:].rearrange("p (h c) -> p h c", h=3)
            nc.gpsimd.tensor_scalar_mul(dst, src, rs_nat[:, si:si + 1])

        def qvn_tiles(si_lo, si_hi):
            for si in range(si_lo, si_hi):
                qvn_tile(si)

        # ---------- attention (j-outer, per-head pipelined) ----------
        jorder = [0, 2, 3, 1]
        bnc_in = [P_dram.tile([2, C, 512], bf16, tag=f"d_in{q}",
                              name=f"bnc_in{q}") for q in range(2)]
        bnc_out = [P_dram.tile([C, 512], bf16, tag=f"d_out{q}",
                               name=f"bnc_out{q}") for q in range(2)]
        KT = [(KT01, slice(0, 64)), (KT01, slice(64, 128)),
              (KT22, slice(0, 64))]
        QVT = [(QVT01, slice(0, 64)), (QVT01, slice(64, 128)),
               (QVT22, slice(0, 64))]
        attnT = [P_at.tile([64, T], bf16, tag="at", bufs=3, name=f"attnT_{h}")
                 for h in range(3)]

        def ln2_half(q):
            # x2 = bnc_out + bproj + xh   (bf16)
            stt_eng = nc.gpsimd if q == 0 else nc.vector
            x2 = []
            for g in range(CT):
                rsg = P_sc.tile([128, 512], bf16, tag="rsg",
                                name=f"rsg_{q}_{g}")
                nc.gpsimd.dma_start(rsg[:],
                                    bnc_out[q][128 * g:128 * (g + 1), :])
                t = P_x2.tile([128, 512], bf16, tag="x2",
                              name=f"x2_{q}_{g}")
                stt_eng.tensor_tensor(t[:], rsg[:], xh_all[3 * q + g][:],
                                      ts.add)
                x2.append(t)
            # LN2 stats
            mu_ps = P_ps_m.tile([1, 512], f32, tag="psm")
            m2_ps = P_ps_m.tile([1, 512], f32, tag="psm")
            for i in range(CT):
                sq = P_sc.tile([128, 512], bf16, tag="sq")
                TT(sq[:], x2[i][:], x2[i][:], ts.mult)
                MM(mu_ps[:], mw[:], x2[i][:],
                   start=(i == 0), stop=(i == CT - 1))
                MM(m2_ps[:], mw[:], sq[:],
                   start=(i == 0), stop=(i == CT - 1))
            mu2r = P_rows.tile([1, 512], bf16, tag=f"mu2r_{q}")
            v2r = P_rows.tile([1, 512], f32, tag=f"v2r_{q}")
            rs2r = P_rows.tile([1, 512], bf16, tag=f"rs2r_{q}")
            nc.scalar.copy(mu2r[:], mu_ps[:])
            nc.vector.tensor_copy(v2r[:], m2_ps[:])
            tmp = P_rows.tile([1, 512], f32, tag=f"tmp2_{q}")
            TT(tmp[:], mu2r[:], mu2r[:], ts.mult)
            TT(v2r[:], v2r[:], tmp[:], ts.subtract)
            nc.scalar.activation(v2r[:], v2r[:], AF.Ln, bias=EPS)
            nc.scalar.activation(rs2r[:], v2r[:], AF.Exp, scale=-0.5)
            mu2b = P_ps_m.tile([128, 512], f32, tag="psm")
            rs2b = P_ps_m.tile([128, 512], f32, tag="psm")
            MM(mu2b[:], ones128, mu2r[:], start=True, stop=True)
            MM(rs2b[:], ones128, rs2r[:], start=True, stop=True)
            h2 = []
            for i in range(CT):
                t = P_h2.tile([128, 512], bf16, tag="h2t",
                              name=f"h2_{q}_{i}")
                TT(t[:], x2[i][:], mu2b[:], ts.subtract)
                TT(t[:], t[:], rs2b[:], ts.mult)
                h2.append(t)
            return x2, h2

        def ffn_half(q, x2, h2):
            qs = slice(512 * q, 512 * (q + 1))
            y_ps = [P_ps_o.tile([128, 512], f32, tag="pso",
                                name=f"y2_ps_{q}_{g}")
                    for g in range(CT)]
            pend_h1 = []
            for mt in range(FF // 128):
                ps = P_ps_m.tile([128, 512], f32, tag="psm")
                for i in range(CT):
                    MM(ps[:], wff1[i][:, 128 * mt:128 * (mt + 1)], h2[i][:],
                       start=(i == 0), stop=(i == CT - 1))
                if len(pend_h1) >= 2:
                    p_mt, p_h1 = pend_h1.pop(0)
                    for g in range(CT):
                        MM(y_ps[g][:],
                           wff2[p_mt][:, 128 * g:128 * (g + 1)], p_h1[:],
                           start=(p_mt == 0), stop=False)
                h1t = P_h1.tile([128, 512], bf16, tag="h1",
                                name=f"h1_{q}_{mt}")
                # q==0 runs concurrently with the RS#2 collective wait that
                # sits on the Pool sequencer -- keep its relus off Pool.
                r = mt % 3 if q else mt % 2
                if r == 0:
                    nc.scalar.activation(h1t[:], ps[:], AF.Relu,
                                         bias=bff1[:, mt:mt + 1])
                elif r == 1:
                    nc.vector.tensor_scalar(h1t[:], ps[:],
                                            bff1[:, mt:mt + 1], 0.0,
                                            ts.add, ts.max)
                else:
                    nc.gpsimd.tensor_scalar(h1t[:], ps[:],
                                            bff1[:, mt:mt + 1], 0.0,
                                            ts.add, ts.max)
                pend_h1.append((mt, h1t))
            while pend_h1:
                p_mt, p_h1 = pend_h1.pop(0)
                for g in range(CT):
                    MM(y_ps[g][:], wff2[p_mt][:, 128 * g:128 * (g + 1)],
                       p_h1[:], start=(p_mt == 0),
                       stop=(p_mt == FF // 128 - 1))
            for g in range(CT):
                ot = P_sc.tile([128, 512], f32, tag="ot")
                nc.vector.scalar_tensor_tensor(
                    ot[:], y_ps[g][:], bff2[:, g:g + 1], x2[g][:],
                    ts.add, ts.add)
                nc.sync.dma_start(d_out[128 * g:128 * (g + 1), qs], ot[:])

        # flat pipelined attention stream: pend entries cross chunk
        # boundaries; each chunk's normalize/proj/RS is emitted as soon as
        # its last attn@V has been issued; per-head normalize frees the
        # o_ps banks incrementally for the next chunk.  K/QV/stats work for
        # later chunks is chopped into units drained one-per-step so the PE
        # fills the exp-latency bubbles instead of ever blocking on a
        # monolithic prefix.
        units = [lambda: stats_chunk(2), lambda: kqv_rsbc(2)]
        units += [lambda i=i: kqv_tile(2, i) for i in range(4)]
        units.append(lambda: stats_chunk(3))
        units.append(lambda: stats_chunk(1))
        units.append(lambda: kqv_rsbc(1))
        units += [lambda i=i: kqv_tile(1, i) for i in range(4)]
        units += [lambda si=si: qvn_tile(si) for si in range(4, 12)]
        units.append(lambda: kqv_rsbc(3))
        units += [lambda i=i: kqv_tile(3, i) for i in range(4)]
        units += [lambda si=si: qvn_tile(si) for si in range(12, 16)]
        req = {0: 0, 2: 6, 3: 26, 1: len(units)}
        upos = [0]

        def drain(upto):
            while upos[0] < min(upto, len(units)):
                units[upos[0]]()
                upos[0] += 1

        PEND_D = 3
        o_ps_map = {}
        ndone = {}

        def normalize(j, h):
            cs = slice(512 * j, 512 * (j + 1))
            rc = P_rc.tile([128, 512], bf16, tag="rcp", bufs=3,
                           name=f"rc_{j}_{h}")
            nc.vector.reciprocal(rc[64:65, :], o_ps_map[j][h][64:65, :])
            rb = P_ps_m.tile([64, 512], f32, tag="psm")
            MM(rb[:], onesT[64:65, 0:64], rc[64:65, :], start=True, stop=True)
            rbs = P_rc.tile([64, 512], f32, tag="rbs", bufs=3)
            nc.scalar.copy(rbs[:], rb[:])
            TT(attnT[h][:, cs], o_ps_map[j][h][0:64, :], rbs[:], ts.mult)

        def proj_rs(j, jx):
            cs = slice(512 * j, 512 * (j + 1))
            for mt in range(CT):
                psp = P_ps_m.tile([128, 512], f32, tag="psm")
                for h in range(3):
                    MM(psp[:], wp[h][:, 128 * mt:128 * (mt + 1)],
                       attnT[h][:, cs], start=(h == 0), stop=(h == 2))
                ysb = P_sc.tile([128, 512], bf16, tag="ysb")
                if jx == 3:
                    nc.scalar.copy(ysb[:], psp[:])
                else:
                    nc.gpsimd.tensor_copy(ysb[:], psp[:])
                nc.sync.dma_start(
                    bnc_in[j % 2][j // 2, 128 * mt:128 * (mt + 1), :],
                    ysb[:])
            if jx == 1:
                rs_collective(0)

        def rs_collective(grp):
            nc.gpsimd.collective_compute(
                "ReduceScatter", mybir.AluOpType.add,
                replica_groups=[[0, 1], [2, 3], [4, 5], [6, 7]],
                ins=[bnc_in[grp].opt()],
                outs=[bnc_out[grp].opt()])

        def attnv(ent):
            p_j, p_si, p_h, p_c0, p_w, p_es = ent
            last = (p_si == 4 * p_j + 3)
            MM(o_ps_map[p_j][p_h][:, p_c0 - 512 * p_j:512],
               qvn[:, 288 * p_si + 96 * p_h:288 * p_si + 96 * (p_h + 1)],
               p_es[:, 0:p_w],
               start=(p_si == 0), stop=last)
            if last:
                normalize(p_j, p_h)
                ndone[p_j] = ndone.get(p_j, 0) + 1
                if ndone[p_j] == 3:
                    proj_rs(p_j, jorder.index(p_j))

        stats_chunk(0)
        kqv_chunk(0)
        qvn_tiles(0, 4)
        pend = []
        nstep = 0
        for jx, j in enumerate(jorder):
            drain(req[j])
            o_ps_map[j] = [P_ps_o.tile([96, 512], f32, tag="pso",
                                       name=f"o_ps_{j}_{h}")
                           for h in range(3)]
            for si in range(4 * j + 4):
                diag = (si // 4 == j)
                c0 = max(512 * j, 128 * si)
                w = 512 * (j + 1) - c0
                for h in range(3):
                    KTt, kp = KT[h]
                    QVTt, qp = QVT[h]
                    s3 = P_ps_s.tile([128, 512], f32, tag="ps_s")
                    MM(s3[:, 0:w],
                       KTt[kp, 128 * si:128 * (si + 1)],
                       QVTt[qp, c0:512 * (j + 1)],
                       start=True, stop=True)
                    if len(pend) >= PEND_D:
                        attnv(pend.pop(0))
                    es = P_es.tile([128, 512], bf16, tag="es")
                    nc.scalar.activation(es[:, 0:w], s3[:, 0:w], AF.Exp,
                                         scale=rsS_nat[:, si:si + 1])
                    if diag:
                        TT(es[:, 0:128], es[:, 0:128], mask[:], ts.mult)
                    pend.append((j, si, h, c0, w, es))
                    nstep += 1
                    if nstep % 2 == 0:
                        drain(nstep // 2)
        while pend:
            attnv(pend.pop(0))
        with tc.tile_wait_until(0.5):
            x2_0, h2_0 = ln2_half(0)
        with tc.tile_wait_until(0.55):
            rs_collective(1)
            ffn_half(0, x2_0, h2_0)
        with tc.tile_wait_until(0.6):
            x2_1, h2_1 = ln2_half(1)
            ffn_half(1, x2_1, h2_1)
    nc.compile()
    return nc


def _shard(inputs):
    x = np.asarray(inputs["x"], np.float32)
    g1 = np.asarray(inputs["ln1_g"], np.float32)
    b1 = np.asarray(inputs["ln1_b"], np.float32)
    wk = np.asarray(inputs["wk"], np.float32)
    wv = np.asarray(inputs["wv"], np.float32)
    wp = np.asarray(inputs["w_proj"], np.float32)
    bp = np.asarray(inputs["b_proj"], np.float32)
    g2 = np.asarray(inputs["ln2_g"], np.float32)
    b2 = np.asarray(inputs["ln2_b"], np.float32)
    wf1 = np.asarray(inputs["w_ff1"], np.float32)
    bf1 = np.asarray(inputs["b_ff1"], np.float32)
    wf2 = np.asarray(inputs["w_ff2"], np.float32)
    bf2 = np.asarray(inputs["b_ff2"], np.float32)

    wkg = wk * g1[None, :, None]       # fold ln1 gain
    wvg = wv * g1[None, :, None]
    vbk = b1 @ wk                      # [NH, HD] ln1-bias contributions
    vbv = b1 @ wv
    wf1g = wf1 * g2[:, None]
    bff1_eff = b2 @ wf1 + bf1

    import ml_dtypes
    bf16 = ml_dtypes.bfloat16
    i, j = np.indices((128, 128))
    mask = np.where(j >= i, 1.0, 0.0).astype(bf16)

    def pack2(a, b):
        return np.ascontiguousarray(np.concatenate([a, b], axis=-1))

    in_maps = []
    for c in range(N_CORES):
        b, hg = c // 2, c % 2
        hs = [3 * hg, 3 * hg + 1, 3 * hg + 2]
        wproj = wp[192 * hg:192 * (hg + 1), :]
        vb_slice = np.concatenate([vbv[h] for h in hs])
        beff = vb_slice @ wproj + bp / 2.0

        wk01 = pack2(wkg[hs[0]], wkg[hs[1]])
        wk22 = pack2(wkg[hs[2]], wkg[hs[2]])
        wv01 = pack2(wvg[hs[0]], wvg[hs[1]])
        wv22 = pack2(wvg[hs[2]], wvg[hs[2]])
        wv3 = np.ascontiguousarray(
            np.concatenate([wvg[h] for h in hs], axis=1))

        def ckb(wpair, vpair):
            return np.ascontiguousarray(
                np.stack([-wpair.sum(0), vpair]))

        m = {
            "xT": np.ascontiguousarray(x[b].T).astype(bf16),
            "xTh": np.ascontiguousarray(
                x[b].T[:, TH * hg:TH * (hg + 1)]
                + beff[:, None]).astype(bf16),
            "wk01": wk01.astype(bf16),
            "wk22": wk22.astype(bf16),
            "wv01": wv01.astype(bf16),
            "wv22": wv22.astype(bf16),
            "wv3": wv3.astype(bf16),
            "ckb01": ckb(wk01, np.concatenate([vbk[hs[0]], vbk[hs[1]]])
                         ).astype(bf16),
            "ckb22": ckb(wk22, np.concatenate([vbk[hs[2]], vbk[hs[2]]])
                         ).astype(bf16),
            "cqb01": ckb(wv01, np.concatenate([vbv[hs[0]], vbv[hs[1]]])
                         ).astype(bf16),
            "cqb22": ckb(wv22, np.concatenate([vbv[hs[2]], vbv[hs[2]]])
                         ).astype(bf16),
            "cqb3": np.ascontiguousarray(-wv3.sum(0))[None, :].astype(bf16),
            "wp0": np.ascontiguousarray(wproj[0:64, :]).astype(bf16),
            "wp1": np.ascontiguousarray(wproj[64:128, :]).astype(bf16),
            "wp2": np.ascontiguousarray(wproj[128:192, :]).astype(bf16),
            "bproj": np.ascontiguousarray(beff.reshape(CT, 128).T),
            "wff1": wf1g.astype(bf16),
            "bff1": np.ascontiguousarray(bff1_eff.reshape(FF // 128, 128).T),
            "wff2": wf2.astype(bf16),
            "bff2": np.ascontiguousarray(bf2.reshape(CT, 128).T),
            "mask": mask,
        }
        in_maps.append(m)
    return in_maps


def kernel(**inputs):
    from concourse.bass_utils import run_bass_kernel_spmd

    if "nc" not in _CACHE:
        _CACHE["nc"] = _build()
    nc = _CACHE["nc"]
    in_maps = _shard(inputs)
    res = run_bass_kernel_spmd(nc, in_maps, list(range(N_CORES)))
    out = np.empty((B, T, C), np.float32)
    for c in range(N_CORES):
        b, hg = c // 2, c % 2
        out[b, TH * hg:TH * (hg + 1), :] = res.results[c]["outT"].T
    return out
